# revision 25
# baseline (speedup 1.0000x reference)
"""Sliding-window GQA attention (T=4096, DIM=2048, H=16, KVH=4, D=128, W=1024)
as an 8-core SPMD Trainium2 Bass/Tile kernel.

Sharding: sequence-parallel. Core c owns queries [512c, 512c+512) and
recomputes K/V for its sliding window (1536 kv slots, zero-padded before
position 0). No collectives.

All matmul operands are bf16 (PSUM accumulation fp32). Weight/x loads are
batched into slab DMAs (a handful of strided descriptors instead of
hundreds of [128,*] tile loads). The attention m-loop is software
pipelined: S[i+2] is issued while exp/mask of S[i+1] runs and Y/den of
S[i] accumulate, and the next head-pair's Q-projection matmuls are woven
between m-steps as tensor-engine gap filler.

Dataflow (everything transposed so softmax needs no cross-partition max):
  Q^T[h] [d=128, q=512]   = RoPE(Wq_h^T x_q^T)        (per head)
  K^T[kvh] [128, 1536]    = RoPE(Wk_kvh^T x_kv^T)
  V[m] [t=128, 512=kvh*d] = x_kv[tile]^T^T ... natural layout per t-tile
  S^T [t-tile, q-span]    = K-tile(stationary) @ Q^T   (PSUM)
  P^T = exp(scale*S^T + kbias[t])   (ACT, bf16 out; kbias kills padded t)
  P^T *= triangle masks on boundary blocks (DVE)
  Y^T[h] += V-tile @ P^T ; den[h] += ones @ P^T        (PSUM accumulate)
  Y^T[h] = Y^T * (1/den)                               (softmax normalize)
  O^T[e-tile] += Wo-chunk(stationary) @ Y^T[h]         -> DRAM [2048, 512]
"""

import math
import os
import sys

import numpy as np
import ml_dtypes

BF16NP = ml_dtypes.bfloat16


def _ensure_paths():
    for p in (
        "/root/.axon_site",
        "/root/.axon_site/_ro/trn_rl_repo",
        "/root/.axon_site/_ro/pypackages",
        "/opt/trn_rl_repo",
        "/opt/pypackages",
    ):
        if os.path.isdir(p) and p not in sys.path:
            sys.path.append(p)


try:
    import concourse.bass as bass  # noqa: F401
except ImportError:
    _ensure_paths()

import concourse.bass as bass
import concourse.mybir as mybir
import concourse.tile as tile
from concourse import bacc
from concourse.bass_utils import run_bass_kernel_spmd

# ---------------------------------------------------------------- constants
N_CORES = 8
T = 4096
DIM = 2048
H = 16
KVH = 4
D = 128
WIN = 1024
ROPE_BASE = 10000.0

TQ = T // N_CORES          # 512 queries per core
TKV = TQ + WIN             # 1536 kv slots per core
NMT = TKV // 128           # 12 kv tiles of 128
NCC = DIM // 128           # 16 contraction chunks
SCALE = 1.0 / math.sqrt(D)
GQ = H // KVH              # 4 q heads per kv head

F32 = mybir.dt.float32
BF16 = mybir.dt.bfloat16

# per kv-tile m: (qlo, qhi) span of local queries it can interact with
SPANS = {
    0: (0, 256), 1: (0, 256), 2: (0, 384), 3: (0, 512),
    4: (0, 512), 5: (0, 512), 6: (0, 512), 7: (0, 512),
    8: (0, 512), 9: (128, 512), 10: (256, 512), 11: (256, 512),
}
# per kv-tile m: (mask_name, lo, hi, zero_lo, zero_hi) in absolute q coords
MASKS = {
    0: ("maskB", 0, 128, 128, 256), 1: ("maskB", 128, 256, None, None),
    2: ("maskB", 256, 384, None, None), 3: ("maskB", 384, 512, None, None),
    4: None, 5: None, 6: None, 7: None,
    8: ("maskA", 0, 128, None, None), 9: ("maskA", 128, 256, None, None),
    10: ("maskA", 256, 384, None, None), 11: ("maskA", 384, 512, 256, 384),
}
# PSUM accumulation order: m=4 first (full-width span -> start=True clears
# the whole Y/den bank), m=11 last (stop=True).
M_ORDER = [4, 5, 6, 7, 0, 1, 2, 3, 8, 9, 10, 11]


# ---------------------------------------------------------------- device code
_NC_CACHE = None


def _build():
    global _NC_CACHE
    if _NC_CACHE is not None:
        return _NC_CACHE

    nc = bacc.Bacc("TRN2", target_bir_lowering=False, debug=False,
                   num_devices=N_CORES)

    # DRAM I/O (per-core contents supplied via in_maps)
    xqT = nc.dram_tensor("xqT", [DIM, TQ], BF16, kind="ExternalInput").ap()
    xkvT = nc.dram_tensor("xkvT", [3 * DIM, 512], BF16, kind="ExternalInput").ap()
    wq = nc.dram_tensor("wq", [8 * DIM, 256], BF16, kind="ExternalInput").ap()
    wk = nc.dram_tensor("wk", [DIM, KVH * D], BF16, kind="ExternalInput").ap()
    wv = nc.dram_tensor("wv", [DIM, KVH * D], BF16, kind="ExternalInput").ap()
    wo = nc.dram_tensor("wo", [8 * DIM, 256], BF16, kind="ExternalInput").ap()
    cosq = nc.dram_tensor("cosq", [D, TQ], F32, kind="ExternalInput").ap()
    sinq = nc.dram_tensor("sinq", [D, TQ], F32, kind="ExternalInput").ap()
    cosk = nc.dram_tensor("cosk", [3 * D, 512], F32, kind="ExternalInput").ap()
    sink = nc.dram_tensor("sink", [3 * D, 512], F32, kind="ExternalInput").ap()
    kbias = nc.dram_tensor("kbias", [128, NMT], F32, kind="ExternalInput").ap()
    maskB = nc.dram_tensor("maskB", [128, 128], BF16, kind="ExternalInput").ap()
    maskA = nc.dram_tensor("maskA", [128, 128], BF16, kind="ExternalInput").ap()
    rotp = nc.dram_tensor("rotp", [128, 128], BF16, kind="ExternalInput").ap()
    ones = nc.dram_tensor("ones", [128, 128], BF16, kind="ExternalInput").ap()
    outT = nc.dram_tensor("outT", [DIM, TQ], BF16, kind="ExternalOutput").ap()

    mask_dram = {"maskB": maskB, "maskA": maskA}

    with tile.TileContext(nc) as tc:
        _emit(nc, tc, xqT, xkvT, wq, wk, wv, wo, cosq, sinq, cosk, sink,
              kbias, mask_dram, rotp, ones, outT)

    nc.compile()
    _NC_CACHE = nc
    return nc


def _emit(nc, tc, xqT, xkvT, wq, wk, wv, wo, cosq, sinq, cosk, sink,
          kbias, mask_dram, rotp, ones, outT):
    from contextlib import ExitStack

    def slabify(dram_slice):
        """[(c p) w] DRAM slice -> [p, c, w] AP for one slab DMA."""
        return dram_slice.rearrange("(c p) w -> p c w", p=128)

    def slab_dst(tile_ap, c, w):
        """[128, c*w] SBUF tile -> [p, c, w] AP matching slabify order."""
        return tile_ap.rearrange("p (c w) -> p c w", c=c, w=w)

    ctx = ExitStack()
    with ctx:
        # pools
        consts = ctx.enter_context(tc.tile_pool(name="consts", bufs=1))
        xkvp = ctx.enter_context(tc.tile_pool(name="xkvp", bufs=8))
        xqp = ctx.enter_context(tc.tile_pool(name="xqp", bufs=4))
        wqp = ctx.enter_context(tc.tile_pool(name="wqp", bufs=2))
        wkp = ctx.enter_context(tc.tile_pool(name="wkp", bufs=4))
        wvp = ctx.enter_context(tc.tile_pool(name="wvp", bufs=4))
        wop = ctx.enter_context(tc.tile_pool(name="wop", bufs=3))
        cskp = ctx.enter_context(tc.tile_pool(name="cskp", bufs=4))
        qtp = ctx.enter_context(tc.tile_pool(name="qtp", bufs=4))
        ktp = ctx.enter_context(tc.tile_pool(name="ktp", bufs=KVH))
        vp = ctx.enter_context(tc.tile_pool(name="vp", bufs=NMT))
        ytp = ctx.enter_context(tc.tile_pool(name="ytp", bufs=H))
        pp = ctx.enter_context(tc.tile_pool(name="pp", bufs=4))
        tmp = ctx.enter_context(tc.tile_pool(name="tmp", bufs=2))
        t12 = ctx.enter_context(tc.tile_pool(name="t12", bufs=3))
        fin = ctx.enter_context(tc.tile_pool(name="fin", bufs=2))
        ps_a = ctx.enter_context(tc.tile_pool(name="ps_a", bufs=2, space="PSUM"))
        ps_b = ctx.enter_context(tc.tile_pool(name="ps_b", bufs=2, space="PSUM"))
        ps_s = ctx.enter_context(tc.tile_pool(name="ps_s", bufs=2, space="PSUM"))
        ps_y = ctx.enter_context(tc.tile_pool(name="ps_y", bufs=2, space="PSUM"))

        Exp = mybir.ActivationFunctionType.Exp

        # ---- constants into SBUF (scalar-engine DMA queue; sync/gpsimd
        # queues carry the x/weight slabs the first matmuls depend on)
        def cload(ap, shape, dtype, tag):
            t = consts.tile(shape, dtype, tag=tag)
            nc.scalar.dma_start(t[:], ap[:])
            return t

        rotp_sb = cload(rotp, [128, 128], BF16, "rotp")
        ones_sb = cload(ones, [128, 128], BF16, "ones")
        kbias_sb = cload(kbias, [128, NMT], F32, "kbias")
        cosq_sb = cload(cosq, [D, TQ], F32, "cosq")
        sinq_sb = cload(sinq, [D, TQ], F32, "sinq")
        mask_sb = {
            name: cload(mask_dram[name], [128, 128], BF16, name)
            for name in ("maskB", "maskA")
        }

        # ---- slab loads (4 contraction chunks of [128,512] per slab)
        wk_slabs = []
        for b in range(4):
            wks = wkp.tile([128, 4 * 512], BF16, tag="wks", name=f"wks{b}")
            nc.gpsimd.dma_start(slab_dst(wks[:], 4, 512), slabify(wk[b * 512:(b + 1) * 512, :]))
            wk_slabs.append(wks)

        def wkc(c, g):
            return wk_slabs[c // 4][:, (c % 4) * 512 + g * 128:
                                    (c % 4) * 512 + (g + 1) * 128]

        xkv_slabs = [[None] * 4 for _ in range(3)]

        def load_xkv_span(s):
            for b in range(4):
                t = xkvp.tile([128, 4 * 512], BF16, tag="xkv",
                              name=f"xkv{s}_{b}")
                nc.sync.dma_start(
                    slab_dst(t[:], 4, 512),
                    slabify(xkvT[s * DIM + b * 512:
                                 s * DIM + (b + 1) * 512, :]))
                xkv_slabs[s][b] = t

        def xsc(s, c):
            return xkv_slabs[s][c // 4][:, (c % 4) * 512:(c % 4 + 1) * 512]

        load_xkv_span(0)

        wv_slabs = []
        for b in range(4):
            wvs = wvp.tile([128, 4 * 512], BF16, tag="wvs", name=f"wvs{b}")
            nc.sync.dma_start(slab_dst(wvs[:], 4, 512), slabify(wv[b * 512:(b + 1) * 512, :]))
            wv_slabs.append(wvs)

        def wvc(c):
            return wv_slabs[c // 4][:, (c % 4) * 512:(c % 4 + 1) * 512]

        xq_slabs = []
        for b in range(4):
            t = xqp.tile([128, 4 * 512], BF16, tag="xq", name=f"xq{b}")
            nc.sync.dma_start(slab_dst(t[:], 4, 512), slabify(xqT[b * 512:(b + 1) * 512, :]))
            xq_slabs.append(t)

        def xqc(c):
            return xq_slabs[c // 4][:, (c % 4) * 512:(c % 4 + 1) * 512]

        load_xkv_span(1)
        load_xkv_span(2)

        def rope(src_ps, sin_sl, cos_sl, dst_ap, width, r_pool, r_tag):
            """dst = src*cos + rot_half(src)*sin  (dst bf16)."""
            s_sb = tmp.tile([128, 512], BF16, tag="ropesb")
            nc.vector.tensor_copy(s_sb[:, :width], src_ps[:, :width])
            r_ps = r_pool.tile([128, 512], F32, tag=r_tag)
            nc.tensor.matmul(r_ps[:, :width], rotp_sb[:], s_sb[:, :width],
                             start=True, stop=True)
            t1 = t12.tile([128, 512], F32, tag="t12")
            nc.vector.tensor_mul(t1[:, :width], r_ps[:, :width], sin_sl)
            t2 = t12.tile([128, 512], F32, tag="t12")
            nc.vector.tensor_mul(t2[:, :width], src_ps[:, :width], cos_sl)
            nc.vector.tensor_add(dst_ap, t1[:, :width], t2[:, :width])

        # ---- Q-projection pair steps (PE gap filler during attention)
        qts = {}

        def make_proj_steps(p_):
            """Issue pair p_'s weight slab DMA now; return PE-work steps."""
            h0 = 2 * p_
            slab = wqp.tile([128, NCC * 256], BF16, tag="wq",
                            name=f"wqs{p_}")
            nc.gpsimd.dma_start(
                slab_dst(slab[:], NCC, 256),
                slabify(wq[p_ * DIM:(p_ + 1) * DIM, :]))
            qpair = [ps_a.tile([128, 512], F32, tag="ps_a",
                               name=f"qps{h0}_{j}") for j in range(2)]
            steps = []

            def c_step(c):
                def go():
                    for j in range(2):
                        nc.tensor.matmul(
                            qpair[j][:],
                            slab[:, c * 256 + j * 128:c * 256 + (j + 1) * 128],
                            xqc(c), start=(c == 0), stop=(c == NCC - 1))
                return go

            for c in range(NCC):
                steps.append(c_step(c))

            def rope_step(j):
                def go():
                    qtj = qtp.tile([128, TQ], BF16, tag="qt",
                                   name=f"qt{h0 + j}")
                    rope(qpair[j], sinq_sb[:], cosq_sb[:], qtj[:], TQ,
                         ps_s, "ps_s")
                    qts[h0 + j] = qtj
                return go

            steps.append(rope_step(0))
            steps.append(rope_step(1))
            return steps

        # ---- phase A: K^T (RoPE'd) and V over 3 spans of 512 kv slots.
        # Pairs 0 and 1 of the Q projection are emitted inside phase A so
        # attention can start with a primed filler pipeline.
        kt_sb = [ktp.tile([128, TKV], BF16, tag="kt", name=f"kt{g}")
                 for g in range(KVH)]
        v_sb = [vp.tile([128, 512], BF16, tag="v", name=f"v{m}")
                for m in range(NMT)]

        for s in range(3):
            cosk_s = cskp.tile([128, 512], F32, tag="csk")
            nc.scalar.dma_start(cosk_s[:], cosk[s * 128:(s + 1) * 128, :])
            sink_s = cskp.tile([128, 512], F32, tag="csk")
            nc.scalar.dma_start(sink_s[:], sink[s * 128:(s + 1) * 128, :])

            # K^T projection: c-outer across 4 psum banks
            kps = [ps_s.tile([128, 512], F32, tag="ps_s", name=f"kps{s}_0"),
                   ps_s.tile([128, 512], F32, tag="ps_s", name=f"kps{s}_1"),
                   ps_y.tile([128, 512], F32, tag="ps_y", name=f"kps{s}_2"),
                   ps_y.tile([128, 512], F32, tag="ps_y", name=f"kps{s}_3")]
            for c in range(NCC):
                for g in range(KVH):
                    nc.tensor.matmul(kps[g][:], wkc(c, g), xsc(s, c),
                                     start=(c == 0), stop=(c == NCC - 1))
            for g in range(KVH):
                rope(kps[g], sink_s[:], cosk_s[:],
                     kt_sb[g][:, s * 512:(s + 1) * 512], 512, ps_b, "ps_b")

            # V projection (natural layout): c-outer across 4 psum banks
            vps = [ps_a.tile([128, 512], F32, tag="ps_a", name=f"vps{s}_0"),
                   ps_a.tile([128, 512], F32, tag="ps_a", name=f"vps{s}_1"),
                   ps_b.tile([128, 512], F32, tag="ps_b", name=f"vps{s}_2"),
                   ps_b.tile([128, 512], F32, tag="ps_b", name=f"vps{s}_3")]
            for c in range(NCC):
                for tt in range(4):
                    nc.tensor.matmul(
                        vps[tt][:],
                        xsc(s, c)[:, tt * 128:(tt + 1) * 128],
                        wvc(c),
                        start=(c == 0), stop=(c == NCC - 1))
            for tt in range(4):
                nc.scalar.copy(v_sb[4 * s + tt][:], vps[tt][:])

            # embed the first two Q-projection pairs into phase A
            if s < 2:
                for step in make_proj_steps(s):
                    step()

        yt_sb = [ytp.tile([128, TQ], BF16, tag="yt", name=f"yt{h}")
                 for h in range(H)]

        wo_slabs = [None] * (NCC // 2)

        def load_wo_slab(np_):
            t = wop.tile([128, H * 256], BF16, tag="wo", name=f"wos{np_}")
            nc.gpsimd.dma_start(slab_dst(t[:], H, 256),
                                slabify(wo[np_ * DIM:(np_ + 1) * DIM, :]))
            wo_slabs[np_] = t

        # ---- attention per head, with S-pipeline and proj filler
        def emit_attn(h, filler):
            g = h // GQ
            qt = qts[h]
            yps = ps_y.tile([128, TQ], F32, tag="ps_y", name=f"yps{h}")
            dps = ps_b.tile([128, TQ], F32, tag="ps_b", name=f"dps{h}")
            pq = {}

            def emit_S(i):
                m = M_ORDER[i]
                qlo, qhi = SPANS[m]
                w = qhi - qlo
                sps = ps_s.tile([128, 512], F32, tag="ps_s",
                                name=f"sps{h}_{m}")
                nc.tensor.matmul(sps[:, :w],
                                 kt_sb[g][:, m * 128:(m + 1) * 128],
                                 qt[:, qlo:qhi], start=True, stop=True)
                p = pp.tile([128, 512], BF16, tag="p", name=f"p{h}_{m}")
                nc.scalar.activation(p[:, :w], sps[:, :w], Exp,
                                     bias=kbias_sb[:, m:m + 1], scale=SCALE)
                mk = MASKS[m]
                if mk is not None:
                    name, lo, hi, zlo, zhi = mk
                    nc.vector.tensor_mul(p[:, lo - qlo:hi - qlo],
                                         p[:, lo - qlo:hi - qlo],
                                         mask_sb[name][:])
                    if zlo is not None:
                        nc.vector.tensor_scalar_mul(
                            p[:, zlo - qlo:zhi - qlo],
                            p[:, zlo - qlo:zhi - qlo], 0.0)
                pq[i] = p

            emit_S(0)
            emit_S(1)
            for i in range(len(M_ORDER)):
                if filler is not None:
                    step = next(filler, None)
                    if step is not None:
                        step()
                m = M_ORDER[i]
                qlo, qhi = SPANS[m]
                w = qhi - qlo
                p = pq.pop(i)
                first = i == 0
                last = i == len(M_ORDER) - 1
                nc.tensor.matmul(yps[:, qlo:qhi],
                                 v_sb[m][:, g * 128:(g + 1) * 128],
                                 p[:, :w], start=first, stop=last)
                nc.tensor.matmul(dps[:, qlo:qhi], ones_sb[:], p[:, :w],
                                 start=first, stop=last)
                if i + 2 < len(M_ORDER):
                    emit_S(i + 2)

            rcp = fin.tile([128, TQ], F32, tag="rcp", name=f"rcp{h}")
            nc.vector.reciprocal_approx_fast(rcp[:], dps[:])
            nc.vector.tensor_mul(yt_sb[h][:], yps[:], rcp[:])

        # O-projection chain for e-pair 0 doubles as PE filler during the
        # last two attention pairs (no projection pairs left to weave in).
        o0_state = {}

        def make_o0_steps():
            slab = wop.tile([128, H * 256], BF16, tag="wo", name="wos0")
            nc.gpsimd.dma_start(slab_dst(slab[:], H, 256),
                                slabify(wo[0:DIM, :]))
            opair = [ps_a.tile([128, 512], F32, tag="ps_a",
                               name=f"ops0_{j}") for j in range(2)]
            o0_state["opair"] = opair

            def h_step(h):
                def go():
                    for j in range(2):
                        nc.tensor.matmul(
                            opair[j][:],
                            slab[:, h * 256 + j * 128:h * 256 + (j + 1) * 128],
                            yt_sb[h][:], start=(h == 0), stop=(h == H - 1))
                return go

            return [h_step(h) for h in range(H)]

        def spaced(steps, gap):
            for st in steps:
                yield st
                for _ in range(gap):
                    yield None

        o0_steps = None
        for p_ in range(H // 2):
            if p_ + 2 < H // 2:
                filler = iter(make_proj_steps(p_ + 2))
            elif o0_steps is None:
                o0_steps = make_o0_steps()
                filler = spaced(o0_steps[0:12], 1)
            else:
                load_wo_slab(1)
                load_wo_slab(2)
                filler = spaced(o0_steps[12:14], 8)
            emit_attn(2 * p_, filler)
            emit_attn(2 * p_ + 1, filler)
            if filler is not None:
                for step in filler:
                    if step is not None:
                        step()

        # ---- phase D: O^T projection in e-tile pairs (slab weight loads,
        # issued two pairs ahead; opair banks alternate between two pools)
        # finish chain 0 (h=14,15 ran as filler already emitted above? no:
        # steps 14,15 still pending) and store its outputs
        for st in o0_steps[14:16]:
            st()
        for j in range(2):
            osb = fin.tile([128, TQ], BF16, tag="osb")
            nc.scalar.copy(osb[:], o0_state["opair"][j][:])
            nc.sync.dma_start(outT[j * 128:(j + 1) * 128, :], osb[:])

        for n0 in range(2, NCC, 2):
            np_ = n0 // 2
            if np_ + 2 < NCC // 2:
                load_wo_slab(np_ + 2)
            opool = ps_a if np_ % 2 == 0 else ps_y
            otag = "ps_a" if np_ % 2 == 0 else "ps_y"
            oslab = wo_slabs[np_]
            opair = [opool.tile([128, 512], F32, tag=otag,
                                name=f"ops{n0}_{j}") for j in range(2)]
            for h in range(H):
                for j in range(2):
                    nc.tensor.matmul(
                        opair[j][:],
                        oslab[:, h * 256 + j * 128:h * 256 + (j + 1) * 128],
                        yt_sb[h][:],
                        start=(h == 0), stop=(h == H - 1))
            for j in range(2):
                osb = fin.tile([128, TQ], BF16, tag="osb")
                nc.scalar.copy(osb[:], opair[j][:])
                nc.sync.dma_start(outT[(n0 + j) * 128:(n0 + j + 1) * 128, :],
                                  osb[:])


# ---------------------------------------------------------------- host side
def _bf16(a):
    return np.ascontiguousarray(np.asarray(a, dtype=np.float32)).astype(BF16NP)


def _host_inputs(x, Wq, Wk, Wv, Wo):
    x = np.asarray(x, dtype=np.float32).reshape(T, DIM)

    inv_freq = 1.0 / (ROPE_BASE ** (np.arange(0, D, 2, dtype=np.float64) / D))
    dfreq = np.concatenate([inv_freq, inv_freq])  # [128] per-dim freq

    wq_b = _bf16(
        np.asarray(Wq).reshape(DIM, 8, 256).transpose(1, 0, 2).reshape(8 * DIM, 256))
    wk_b = _bf16(Wk)
    wv_b = _bf16(Wv)
    wo_b = _bf16(
        np.asarray(Wo).reshape(DIM, 8, 256).transpose(1, 0, 2).reshape(8 * DIM, 256))

    u = np.arange(128)[:, None]
    maskB = _bf16((np.arange(128)[None, :] < u).astype(np.float32))
    maskA = _bf16((u <= np.arange(128)[None, :]).astype(np.float32))

    rotp = np.zeros((128, 128), np.float32)
    d = np.arange(128)
    rotp[(d + 64) % 128, d] = 1.0  # out[d] = in[(d+64)%128]

    ones = np.ones((128, 128), np.float32)

    in_maps = []
    for c in range(N_CORES):
        qs = c * TQ
        xq = x[qs:qs + TQ]                      # [512, 2048]
        xkv = np.zeros((TKV, DIM), np.float32)  # [1536, 2048]
        lo = qs - WIN
        src_lo = max(0, lo)
        xkv[src_lo - lo:TKV] = x[src_lo:qs + TQ]

        pos_q = np.arange(qs, qs + TQ, dtype=np.float64)
        pos_k = np.arange(lo, qs + TQ, dtype=np.float64)
        angq = dfreq[:, None] * pos_q[None, :]  # [128, 512]
        angk = dfreq[:, None] * pos_k[None, :]  # [128, 1536]
        sgn = np.where(np.arange(D) < D // 2, -1.0, 1.0)[:, None]

        kb = np.zeros((128, NMT), np.float32)
        for m in range(NMT):
            t_abs = 128 * m + np.arange(128)
            kb[:, m] = np.where(t_abs < WIN - qs, -30.0, 0.0)

        in_maps.append({
            "xqT": _bf16(xq.T),
            "xkvT": _bf16(
                xkv.T.reshape(DIM, 3, 512).transpose(1, 0, 2).reshape(3 * DIM, 512)),
            "wq": wq_b, "wk": wk_b, "wv": wv_b, "wo": wo_b,  # wq/wo pre-paired
            "cosq": np.cos(angq).astype(np.float32),
            "sinq": (sgn * np.sin(angq)).astype(np.float32),
            "cosk": np.ascontiguousarray(np.cos(angk).astype(np.float32)
                .reshape(D, 3, 512).transpose(1, 0, 2)).reshape(3 * D, 512),
            "sink": np.ascontiguousarray(((sgn * np.sin(angk)).astype(np.float32))
                .reshape(D, 3, 512).transpose(1, 0, 2)).reshape(3 * D, 512),
            "kbias": kb,
            "maskB": maskB, "maskA": maskA,
            "rotp": _bf16(rotp),
            "ones": _bf16(ones),
        })
    return in_maps


def kernel(x, Wq, Wk, Wv, Wo, _trace=False, _trace_kwargs=None):
    nc = _build()
    in_maps = _host_inputs(x, Wq, Wk, Wv, Wo)
    res = run_bass_kernel_spmd(nc, in_maps, core_ids=list(range(N_CORES)),
                               trace=_trace, **(_trace_kwargs or {}))
    out = np.empty((1, T, DIM), np.float32)
    for c in range(N_CORES):
        out[0, c * TQ:(c + 1) * TQ, :] = \
            res.results[c]["outT"].astype(np.float32).T
    if _trace:
        kernel.last_results = res
    return out
